# revision 17
# baseline (speedup 1.0000x reference)
"""CASSI GAP reconstruction (DifferentiableGAPTV) on 8 Trainium2 NeuronCores.

Strategy: shard H=512 rows across 8 cores as 128-row slabs (64 output rows +
32-row halo each side).  Rows are independent except the depthwise conv
(3-tap => +-1 row/iter * 12 iters = 12-row dependency), so the halo makes the
whole 12-iteration loop collective-free; each core's central 64 rows are exact.

Numerics (validated vs the fp32 reference on CPU, rel err ~1e-2 < 2e-2):
 - 5-tap sigma=0.5 Gaussian -> renormalized 3-tap (outer taps are 2.6e-4).
 - band states, masks, and per-band elementwise ops in bf16 (DVE 2x mode);
   the measurement-plane accumulator s = y1 + y stays fp32 (updated via
   s' = 0.5*(s + t0) + y since t0 = s - 2*yb, so GPSIMD never touches PSUM).

Per-core, per iteration (bands l = 0..27, dx[l] == l):
  A:  yb = sum_l shift_l(m*x_l)  -- identity matmuls into a PSUM plane, all
      emitted at the END of the iteration (their inputs u_l = m*x_l trickle
      in from DVE/GPSIMD as the copies land); the boundary chain is then
      just "last A-matmul -> t0" and the conv(x) matmuls of the next
      iteration's split bands keep PE busy across it.
  B:  t0 = s - 2*yb (DVE stt, bf16 out)
  C:  v_l = mi_l * t0[l:l+W]  (mi = m/Phi_sum, host-precomputed bf16;
      one DVE op per 4 bands via an overlapping-window AP, dx[l] == l)
      bands 0,1:   x_l' = conv(x_l) + conv(v_l)  (6 matmuls, no w needed)
      bands 2..27: w_l = x_l + v_l (DVE, batched, two quads ahead of PE),
        x_l' = conv(w_l)
      conv = 2D 3x3 via 3 matmuls (row conv in the weights, col taps as
      +-1-shifted rhs windows).  For WING_QUADS the two +-1 col taps are
      folded into one matmul on sw = w<<1 + w>>1 (g3[0] == g3[2]), trading
      213ns of PE for 282ns of DVE per band.
      PSUM->SBUF copies run on ACT; all v/w/sw ops are emitted before the
      conv loop so the in-order DVE queue never head-of-line blocks.
"""
import sys

sys.path.insert(0, "/opt/trn_rl_repo")
import numpy as np
import ml_dtypes
import concourse.bass as bass
import concourse.mybir as mybir
import concourse.tile as tile
from concourse.bass_utils import run_bass_kernel_spmd
from bass_rust import AP

H, W, L = 512, 512, 28
N_ITER = 12
SIGMA = 0.5
PI = 3.141592653589793
NCORES = 8
ROWS = 128          # slab rows per core
OUT_ROWS = 64       # exact output rows per core
HALO = 32           # (ROWS - OUT_ROWS) / 2
WM = W + L - 1      # measurement-plane width (539)
XP = W + 4          # padded band pitch (2 zero cols each side)

f32 = mybir.dt.float32
bf16 = mybir.dt.bfloat16
MUL = mybir.AluOpType.mult
ADD = mybir.AluOpType.add

NPAIR = L // 2                     # 14 band pairs
POOL_U_PAIRS = (0, 1, 2, 3, 4, 5, 6)   # u pairs computed on GPSIMD (their
                                   # copies land earliest, so the slow Pool
                                   # ops finish well before the A block)
WING_QUADS = (3, 4)                # quads using the sw wing-sum conv


def _offsets(s, phi_deg):
    phi = phi_deg * PI / 180.0
    dx = s * np.cos(phi)
    dy = s * np.sin(phi)
    dx = dx - dx.min()
    dy = dy - dy.min()
    return np.rint(dx).astype(np.int32), np.rint(dy).astype(np.int32)


def _gauss3(sigma):
    ksize = max(3, int(6 * sigma + 1) | 1)
    ax = np.arange(ksize, dtype=np.float32) - ksize // 2
    g1 = np.exp(-0.5 * (ax / sigma) ** 2)
    g1 = g1 / g1.sum()
    c = ksize // 2
    g3 = g1[c - 1 : c + 2].astype(np.float64)
    g3 = (g3 / g3.sum()).astype(np.float32)
    return g3  # [3]


def _split_excess_waits(nc, max_w=1):
    """walrus in this toolchain accepts at most one sync wait per instruction;
    hoist excess waits onto preceding same-engine NoOp carriers."""
    ctr = 0
    for f in nc.m.functions:
        for bb in f.blocks:
            il = bb.instructions
            i = 0
            while i < len(il):
                inst = il[i]
                si = inst.sync_info
                w = list(si.on_wait) if (si and si.on_wait) else []
                if len(w) > max_w:
                    si.on_wait = w[-max_w:]
                    extra = w[:-max_w]
                    pos = i
                    for j in range(0, len(extra), max_w):
                        ctr += 1
                        nop = mybir.InstNoOp(
                            name=f"I-waitsplit-{ctr}", ins=[], outs=[]
                        )
                        nop.engine = inst.engine
                        nop.sync_info = mybir.SyncInfo(
                            on_wait=extra[j : j + max_w], on_update=[]
                        )
                        il.insert(pos, nop)
                        pos += 1
                        i += 1
                i += 1


def _win3(tile2d, l0, n, w):
    """[128, n, w] overlapping-window view of a [128, >=l0+n-1+w] tile:
    out[:, j, c] = tile2d[:, l0 + j + c]  (band axis stride = 1 element)."""
    base = tile2d[:, l0 : l0 + w]
    pairs = [list(p) for p in base.ap]
    assert len(pairs) == 2
    return AP(base.tensor, base.offset, [pairs[0], [1, n], [1, w]])


def build_nc(n_iter=N_ITER):
    nc = bass.Bass()
    y_in = nc.declare_dram_parameter("y_slab", [ROWS, WM], f32, isOutput=False)
    m_in = nc.declare_dram_parameter("m_slab", [ROWS, W], bf16, isOutput=False)
    mi_in = nc.declare_dram_parameter("mi_slab", [ROWS, L, W], bf16, isOutput=False)
    w_in = nc.declare_dram_parameter("wmats", [128, 4, 128], bf16, isOutput=False)
    out = nc.declare_dram_parameter("xout", [L, OUT_ROWS, W], f32, isOutput=True)

    with tile.TileContext(nc) as tc:
        with (
            tc.tile_pool(name="state", bufs=1) as st,
            tc.tile_pool(name="ybps", bufs=2, space="PSUM") as ybp,
            tc.tile_pool(name="cps", bufs=2, space="PSUM") as cp,
        ):
            # ---- load inputs (small ones first; mi streams during preamble)
            y_sb = st.tile([ROWS, WM], f32)
            m_sb = st.tile([ROWS, W], bf16)
            wm = st.tile([128, 4, 128], bf16)
            mi = st.tile([ROWS, L, W], bf16)
            nc.sync.dma_start(y_sb[:], y_in[:])
            nc.sync.dma_start(m_sb[:], m_in[:])
            nc.sync.dma_start(wm[:], w_in[:])
            nc.sync.dma_start(mi[:], mi_in[:])

            W_I = wm[:, 0, :]
            W_C = [wm[:, 1 + t, :] for t in range(3)]  # col taps -1, 0, +1

            # ---- persistent state
            ybf = st.tile([ROWS, WM], bf16)
            m2_sb = st.tile([ROWS, W], bf16)
            s_sb = st.tile([ROWS, WM], f32)
            stmp = st.tile([ROWS, WM], f32)
            half = st.tile([ROWS, 1], f32)
            t0_sb = st.tile([ROWS, WM], bf16)
            xs = st.tile([ROWS, L, XP], bf16)
            zr = st.tile([128, L], bf16)
            # w buffers must be distinct per quad: their conv consumers are
            # emitted a whole loop later, so any slot reuse would make the
            # program-order dep tracker bind those convs to the wrong write
            wq = [st.tile([ROWS, 4, XP], bf16, name=f"wq{i}") for i in range(6)]
            NVW = 5
            vq = [st.tile([ROWS, 4, XP], bf16, name=f"vq{i}") for i in range(NVW)]
            sq = [st.tile([ROWS, 4, XP], bf16, name=f"sq{i}") for i in range(2)]
            w0p = st.tile([ROWS, 2, XP], bf16)
            up = [st.tile([ROWS, 2, W], bf16, name=f"up{i}") for i in range(NPAIR)]
            stg = [st.tile([ROWS, 2, W], f32, name=f"stg{i}") for i in range(2)]

            nc.vector.tensor_copy(ybf[:], y_sb[:])
            nc.vector.tensor_mul(out=m2_sb[:], in0=m_sb[:], in1=m_sb[:])
            nc.vector.tensor_scalar_mul(s_sb[:], y_sb[:], 2.0)
            nc.vector.memset(half[:], 0.5)
            nc.vector.memset(zr[:], 0.0)
            zp = st.tile([128, 2], bf16)
            nc.vector.memset(zp[:], 0.0)
            # zero the pad columns once; all later writes stay inside [2, 514)
            for t in (xs, *wq, *vq, *sq, w0p):
                nb = t.shape[1]
                nc.vector.tensor_copy(
                    t[:, :, 0:2], zp[:, None, :].to_broadcast((ROWS, nb, 2))
                )
                nc.vector.tensor_copy(
                    t[:, :, XP - 2 : XP], zp[:, None, :].to_broadcast((ROWS, nb, 2))
                )

            yb_tiles = {}

            def yb_tile(k):
                if k not in yb_tiles:
                    yb_tiles[k] = ybp.tile(
                        [ROWS, WM + 5], f32, tag="yb", name=f"yb{k}"
                    )
                return yb_tiles[k]

            def emit_zero_tail(k):
                nc.tensor.matmul(
                    yb_tile(k)[:, W : W + L], W_I, zr[:],
                    start=True, stop=False, skip_group_check=True,
                )

            def emit_A_band(k, l, u_ap):
                # matmul outs must not cross the PSUM bank boundary at col 512
                yb = yb_tile(k)
                if l == 0:
                    nc.tensor.matmul(
                        yb[:, 0:W], W_I, u_ap,
                        start=True, stop=False, skip_group_check=True,
                    )
                else:
                    nc.tensor.matmul(
                        yb[:, l:W], W_I, u_ap[:, 0 : W - l],
                        start=False, stop=False, skip_group_check=True,
                    )
                    nc.tensor.matmul(
                        yb[:, W : W + l], W_I, u_ap[:, W - l : W],
                        start=False, stop=(l == L - 1), skip_group_check=True,
                    )

            def u_engine(p):
                return nc.gpsimd if p in POOL_U_PAIRS else nc.vector

            # ---- preamble: u0 = (m*m)*y[shift] (one DVE/Pool op per pair),
            # x0 = m*y[shift] (one 28-band DVE op), then the A(0) block
            for p in range(NPAIR):
                eng = nc.gpsimd if p in (0, 2, 4, 6) else nc.vector
                eng.tensor_mul(
                    out=up[p][:],
                    in0=m2_sb[:, None, :].to_broadcast((ROWS, 2, W)),
                    in1=_win3(ybf, 2 * p, 2, W),
                )
            nc.vector.tensor_mul(
                out=xs[:, :, 2 : 2 + W],
                in0=m_sb[:, None, :].to_broadcast((ROWS, L, W)),
                in1=_win3(ybf, 0, L, W),
            )
            emit_zero_tail(0)
            for p in range(NPAIR):
                emit_A_band(0, 2 * p, up[p][:, 0, :])
                emit_A_band(0, 2 * p + 1, up[p][:, 1, :])

            # ---- iterations
            x2_tiles = {}

            def x2_tile(k, p):
                # one PSUM tile per band PAIR (2 banks) so the ACT copy
                # moves both bands in a single 1024-wide instruction
                x2_tiles[(k, p)] = cp.tile(
                    [ROWS, 2, W], f32, tag="x2", name=f"x2_{k}_{p}"
                )
                return x2_tiles[(k, p)]

            def emit_conv_mms(x2, rhs_tile, idx, start, stop, sw_tile=None):
                # x2: [ROWS, W] PSUM region; rhs_tile: [ROWS, nb, XP] holding
                # the band at cols [2, 514)
                nc.tensor.matmul(
                    x2, W_C[1], rhs_tile[:, idx, 2 : 2 + W],
                    start=start, stop=False, skip_group_check=True,
                )
                if sw_tile is not None:
                    # wing taps fused: g3[0]*B @ (w<<1 + w>>1)
                    nc.tensor.matmul(
                        x2, W_C[0], sw_tile[:, idx, 2 : 2 + W],
                        start=False, stop=stop, skip_group_check=True,
                    )
                    return
                nc.tensor.matmul(
                    x2, W_C[0], rhs_tile[:, idx, 1 : 1 + W],
                    start=False, stop=False, skip_group_check=True,
                )
                nc.tensor.matmul(
                    x2, W_C[2], rhs_tile[:, idx, 3 : 3 + W],
                    start=False, stop=stop, skip_group_check=True,
                )

            def emit_copy_pair(k, p, last):
                x2 = x2_tiles[(k, p)]
                l0 = 2 * p
                if last:
                    sg = stg[p % 2]
                    nc.scalar.copy(sg[:], x2[:])
                    for j in range(2):
                        nc.sync.dma_start(
                            out[l0 + j, :, :], sg[HALO : HALO + OUT_ROWS, j, :]
                        )
                else:
                    nc.scalar.copy(xs[:, l0 : l0 + 2, 2 : 2 + W], x2[:])

            def emit_u_pair(k, p):
                # u_l = m * x_l (new xs) -> feeds the yb(k+1) A block
                u_engine(p).tensor_mul(
                    out=up[p][:],
                    in0=m_sb[:, None, :].to_broadcast((ROWS, 2, W)),
                    in1=xs[:, 2 * p : 2 * p + 2, 2 : 2 + W],
                )

            for k in range(n_iter):
                last = k == n_iter - 1
                yb = yb_tile(k)
                # boundary: conv(x) of split bands 0,1 needs no t0 -> PE
                # crosses the A->t0->v0 chain without idling
                x2p0 = x2_tile(k, 0)
                for b in range(2):
                    emit_conv_mms(x2p0[:, b, :], xs, b, start=True, stop=False)
                # B: t0 = s - 2*yb  (bf16 out)
                nc.vector.scalar_tensor_tensor(
                    out=t0_sb[:], in0=yb[:, 0:WM], scalar=-2.0,
                    in1=s_sb[:], op0=MUL, op1=ADD,
                )
                # quad 0: v in two halves; conv + copies interleaved
                nc.vector.tensor_mul(
                    out=vq[0][:, 0:2, 2 : 2 + W],
                    in0=mi[:, 0:2, :],
                    in1=_win3(t0_sb, 0, 2, W),
                )
                for b in range(2):
                    emit_conv_mms(x2p0[:, b, :], vq[0], b, start=False, stop=True)
                emit_copy_pair(k, 0, last)
                nc.vector.tensor_mul(
                    out=vq[0][:, 2:4, 2 : 2 + W],
                    in0=mi[:, 2:4, :],
                    in1=_win3(t0_sb, 2, 2, W),
                )
                nc.vector.tensor_add(
                    out=w0p[:, :, 2 : 2 + W],
                    in0=xs[:, 2:4, 2 : 2 + W],
                    in1=vq[0][:, 2:4, 2 : 2 + W],
                )
                x2p1 = x2_tile(k, 1)
                for b in (2, 3):
                    emit_conv_mms(x2p1[:, b - 2, :], w0p, b - 2, start=True, stop=True)
                emit_copy_pair(k, 1, last)
                # all remaining v/w/sw upfront: the in-order DVE queue stays
                # two-plus quads ahead of the PE conv loop
                for q in range(1, 7):
                    nc.vector.tensor_mul(
                        out=vq[q % NVW][:, 0:4, 2 : 2 + W],
                        in0=mi[:, 4 * q : 4 * q + 4, :],
                        in1=_win3(t0_sb, 4 * q, 4, W),
                    )
                    nc.vector.tensor_add(
                        out=wq[q - 1][:, 0:4, 2 : 2 + W],
                        in0=xs[:, 4 * q : 4 * q + 4, 2 : 2 + W],
                        in1=vq[q % NVW][:, 0:4, 2 : 2 + W],
                    )
                    if q in WING_QUADS:
                        wb = wq[q - 1]
                        nc.vector.tensor_add(
                            out=sq[WING_QUADS.index(q)][:, 0:4, 2 : 2 + W],
                            in0=wb[:, 0:4, 1 : 1 + W],
                            in1=wb[:, 0:4, 3 : 3 + W],
                        )
                # conv loop (PE) + copies (ACT) + Pool u pairs
                for q in range(1, 7):
                    wb = wq[q - 1]
                    sb = sq[WING_QUADS.index(q)] if q in WING_QUADS else None
                    for pp in range(2):
                        p = 2 * q + pp
                        x2p = x2_tile(k, p)
                        for j in range(2):
                            emit_conv_mms(
                                x2p[:, j, :], wb, 2 * pp + j,
                                start=True, stop=True, sw_tile=sb,
                            )
                        emit_copy_pair(k, p, last)
                    if not last and q <= 4:
                        # Pool u pairs, spread so GPSIMD starts early
                        for p in (2 * q - 2, 2 * q - 1):
                            if p in POOL_U_PAIRS:
                                emit_u_pair(k, p)
                if not last:
                    # remaining u pairs on DVE (after all v/w in queue order)
                    for p in range(NPAIR):
                        if p not in POOL_U_PAIRS:
                            emit_u_pair(k, p)
                    # the A(k+1) block: all identity matmuls at the very end
                    # of PE's program order; t0(k+1) follows the last one
                    emit_zero_tail(k + 1)
                    for p in range(NPAIR):
                        emit_A_band(k + 1, 2 * p, up[p][:, 0, :])
                        emit_A_band(k + 1, 2 * p + 1, up[p][:, 1, :])
                    # s' = 0.5*(s + t0) + y  == s + y - yb, on GPSIMD (reads
                    # only SBUF), needed only by t0(k+1)
                    nc.gpsimd.tensor_add(
                        out=stmp[:], in0=s_sb[:], in1=t0_sb[:]
                    )
                    nc.gpsimd.tensor_mul(
                        out=s_sb[:], in0=stmp[:],
                        in1=half[:, 0:1].to_broadcast((ROWS, WM)),
                    )
                    nc.gpsimd.tensor_add(
                        out=s_sb[:], in0=s_sb[:], in1=y_sb[:]
                    )

    _split_excess_waits(nc, max_w=1)
    return nc


def _host_inputs(y_1hw, mask2d):
    y2 = np.asarray(y_1hw, dtype=np.float32)[0]      # [512, 539]
    m2 = np.asarray(mask2d, dtype=np.float32)        # [512, 512]
    g3 = _gauss3(SIGMA)

    # Phi_sum / mi on the full grid (host precompute; Phi depends only on m)
    Phi = np.zeros((H, WM), dtype=np.float32)
    for l in range(L):
        Phi[:, l : l + W] += m2
    Phi = np.maximum(Phi, 1.0)
    invPhi = (1.0 / Phi).astype(np.float32)

    ident = np.eye(128, dtype=np.float32)

    in_maps = []
    for c in range(NCORES):
        rk = 64 * c - HALO
        y_slab = np.zeros((ROWS, WM), dtype=np.float32)
        m_slab = np.zeros((ROWS, W), dtype=np.float32)
        mi_slab = np.zeros((ROWS, L, W), dtype=np.float32)
        lo = max(0, -rk)              # first valid slab row
        hi = min(ROWS, H - rk)        # one past last valid slab row
        y_slab[lo:hi] = y2[rk + lo : rk + hi]
        m_slab[lo:hi] = m2[rk + lo : rk + hi]
        iv = invPhi[rk + lo : rk + hi]  # [vr, WM]
        for l in range(L):
            mi_slab[lo:hi, l, :] = m_slab[lo:hi] * iv[:, l : l + W]
        # banded 3-tap row-conv matrix, zeroed outside the valid row range
        B = np.zeros((128, 128), dtype=np.float32)
        for kk in range(-1, 2):
            for i in range(128):
                ip = i + kk
                if lo <= i < hi and lo <= ip < hi:
                    B[ip, i] = g3[kk + 1]
        wmats = np.zeros((128, 4, 128), dtype=np.float32)
        wmats[:, 0, :] = ident
        for t in range(3):
            wmats[:, 1 + t, :] = g3[t] * B
        in_maps.append(
            {
                "y_slab": y_slab,
                "m_slab": m_slab.astype(ml_dtypes.bfloat16),
                "mi_slab": mi_slab.astype(ml_dtypes.bfloat16),
                "wmats": wmats.astype(ml_dtypes.bfloat16),
            }
        )
    return in_maps


_NC_CACHE = {}


def _get_nc(dx, n_iter=N_ITER):
    key = (tuple(int(v) for v in dx), n_iter)
    if key not in _NC_CACHE:
        assert all(int(d) == i for i, d in enumerate(key[0])), (
            "kernel assumes dx[l] == l"
        )
        _NC_CACHE[key] = build_nc(n_iter)
    return _NC_CACHE[key]


def kernel(y_1hw, mask2d, phi_d_deg, s_nom, n_iter=N_ITER, trace=False):
    s = np.asarray(s_nom, dtype=np.float32)
    phi = float(np.asarray(phi_d_deg))
    dx, dy = _offsets(s, phi)
    assert (dy == 0).all(), "kernel assumes dy == 0 (row shifts unsupported)"
    nc = _get_nc(dx, n_iter)
    in_maps = _host_inputs(y_1hw, mask2d)
    res = run_bass_kernel_spmd(nc, in_maps, list(range(NCORES)), trace=trace)
    x_full = np.empty((1, L, H, W), dtype=np.float32)
    for c in range(NCORES):
        x_full[0, :, 64 * c : 64 * (c + 1), :] = res.results[c]["xout"]
    kernel.last_results = res
    return x_full


# revision 19
# speedup vs baseline: 1.1258x; 1.1258x over previous
"""CASSI GAP reconstruction (DifferentiableGAPTV) on 8 Trainium2 NeuronCores.

Strategy: shard H=512 rows across 8 cores as 128-row slabs (64 output rows +
32-row halo each side).  Rows are independent except the depthwise conv
(3-tap => +-1 row/iter * 12 iters = 12-row dependency), so the halo makes the
whole 12-iteration loop collective-free; each core's central 64 rows are exact.

Numerics (validated vs the fp32 reference on CPU, rel err ~1e-2 < 2e-2):
 - 5-tap sigma=0.5 Gaussian -> renormalized 3-tap (outer taps are 2.6e-4).
 - band states, masks, and per-band elementwise ops in bf16 (DVE 2x mode);
   the measurement-plane accumulator s = y1 + y stays fp32 (updated via
   s' = 0.5*(s + t0) + y since t0 = s - 2*yb, so GPSIMD never touches PSUM).

Per-core, per iteration (bands l = 0..27, dx[l] == l):
  A:  yb = sum_l shift_l(m*x_l)  -- identity matmuls into a PSUM plane, all
      emitted at the END of the iteration (their inputs u_l = m*x_l trickle
      in from DVE/GPSIMD as the copies land); the boundary chain is then
      just "last A-matmul -> t0" and the conv(x) matmuls of the next
      iteration's split bands keep PE busy across it.
  B:  t0 = s - 2*yb (DVE stt, bf16 out)
  C:  v_l = mi_l * t0[l:l+W]  (mi = m/Phi_sum, host-precomputed bf16;
      one DVE op per 4 bands via an overlapping-window AP, dx[l] == l)
      bands 0,1:   x_l' = conv(x_l) + conv(v_l)  (6 matmuls, no w needed)
      bands 2..27: w_l = x_l + v_l (DVE, batched, two quads ahead of PE),
        x_l' = conv(w_l)
      conv = 2D 3x3 via 3 matmuls (row conv in the weights, col taps as
      +-1-shifted rhs windows).  For WING_QUADS the two +-1 col taps are
      folded into one matmul on sw = w<<1 + w>>1 (g3[0] == g3[2]), trading
      213ns of PE for 282ns of DVE per band.
      PSUM->SBUF copies run on ACT; all v/w/sw ops are emitted before the
      conv loop so the in-order DVE queue never head-of-line blocks.
"""
import sys

sys.path.insert(0, "/opt/trn_rl_repo")
import numpy as np
import ml_dtypes
import concourse.bass as bass
import concourse.mybir as mybir
import concourse.tile as tile
from concourse.bass_utils import run_bass_kernel_spmd
from bass_rust import AP

H, W, L = 512, 512, 28
N_ITER = 12
SIGMA = 0.5
PI = 3.141592653589793
NCORES = 8
ROWS = 128          # slab rows per core
OUT_ROWS = 64       # exact output rows per core
HALO = 32           # (ROWS - OUT_ROWS) / 2
WM = W + L - 1      # measurement-plane width (539)
XP = W + 4          # padded band pitch (2 zero cols each side)

f32 = mybir.dt.float32
bf16 = mybir.dt.bfloat16
MUL = mybir.AluOpType.mult
ADD = mybir.AluOpType.add

NPAIR = L // 2                     # 14 band pairs
POOL_U_PAIRS = (0, 1, 2, 3, 4)   # u pairs computed on GPSIMD (their
                                   # copies land earliest, so the slow Pool
                                   # ops finish well before the A block)
WING_QUADS = (3, 4)                # quads using the sw wing-sum conv


def _offsets(s, phi_deg):
    phi = phi_deg * PI / 180.0
    dx = s * np.cos(phi)
    dy = s * np.sin(phi)
    dx = dx - dx.min()
    dy = dy - dy.min()
    return np.rint(dx).astype(np.int32), np.rint(dy).astype(np.int32)


def _gauss3(sigma):
    ksize = max(3, int(6 * sigma + 1) | 1)
    ax = np.arange(ksize, dtype=np.float32) - ksize // 2
    g1 = np.exp(-0.5 * (ax / sigma) ** 2)
    g1 = g1 / g1.sum()
    c = ksize // 2
    g3 = g1[c - 1 : c + 2].astype(np.float64)
    g3 = (g3 / g3.sum()).astype(np.float32)
    return g3  # [3]


def _split_excess_waits(nc, max_w=1):
    """walrus in this toolchain accepts at most one sync wait per instruction;
    hoist excess waits onto preceding same-engine NoOp carriers."""
    ctr = 0
    for f in nc.m.functions:
        for bb in f.blocks:
            il = bb.instructions
            i = 0
            while i < len(il):
                inst = il[i]
                si = inst.sync_info
                w = list(si.on_wait) if (si and si.on_wait) else []
                if len(w) > max_w:
                    si.on_wait = w[-max_w:]
                    extra = w[:-max_w]
                    pos = i
                    for j in range(0, len(extra), max_w):
                        ctr += 1
                        nop = mybir.InstNoOp(
                            name=f"I-waitsplit-{ctr}", ins=[], outs=[]
                        )
                        nop.engine = inst.engine
                        nop.sync_info = mybir.SyncInfo(
                            on_wait=extra[j : j + max_w], on_update=[]
                        )
                        il.insert(pos, nop)
                        pos += 1
                        i += 1
                i += 1


def _win3(tile2d, l0, n, w):
    """[128, n, w] overlapping-window view of a [128, >=l0+n-1+w] tile:
    out[:, j, c] = tile2d[:, l0 + j + c]  (band axis stride = 1 element)."""
    base = tile2d[:, l0 : l0 + w]
    pairs = [list(p) for p in base.ap]
    assert len(pairs) == 2
    return AP(base.tensor, base.offset, [pairs[0], [1, n], [1, w]])


def build_nc(n_iter=N_ITER):
    nc = bass.Bass()
    y_in = nc.declare_dram_parameter("y_slab", [ROWS, WM], f32, isOutput=False)
    m_in = nc.declare_dram_parameter("m_slab", [ROWS, W], bf16, isOutput=False)
    mi_in = nc.declare_dram_parameter("mi_slab", [ROWS, L, W], bf16, isOutput=False)
    w_in = nc.declare_dram_parameter("wmats", [128, 4, 128], bf16, isOutput=False)
    out = nc.declare_dram_parameter("xout", [L, OUT_ROWS, W], f32, isOutput=True)

    with tile.TileContext(nc) as tc:
        with (
            tc.tile_pool(name="state", bufs=1) as st,
            tc.tile_pool(name="ybps", bufs=2, space="PSUM") as ybp,
            tc.tile_pool(name="cps", bufs=4, space="PSUM") as cp,
        ):
            # ---- load inputs (small ones first; mi streams during preamble)
            y_sb = st.tile([ROWS, WM], f32)
            m_sb = st.tile([ROWS, W], bf16)
            wm = st.tile([128, 4, 128], bf16)
            mi = st.tile([ROWS, L, W], bf16)
            nc.sync.dma_start(y_sb[:], y_in[:])
            nc.sync.dma_start(m_sb[:], m_in[:])
            nc.sync.dma_start(wm[:], w_in[:])
            nc.sync.dma_start(mi[:], mi_in[:])

            W_I = wm[:, 0, :]
            W_C = [wm[:, 1 + t, :] for t in range(3)]  # col taps -1, 0, +1

            # ---- persistent state
            ybf = st.tile([ROWS, WM], bf16)
            m2_sb = st.tile([ROWS, W], bf16)
            s_sb = st.tile([ROWS, WM], f32)
            stmp = st.tile([ROWS, WM], f32)
            half = st.tile([ROWS, 1], f32)
            t0_sb = st.tile([ROWS, WM], bf16)
            xs = st.tile([ROWS, L, XP], bf16)
            zr = st.tile([128, L], bf16)
            # w buffers must be distinct per quad: their conv consumers are
            # emitted a whole loop later, so any slot reuse would make the
            # program-order dep tracker bind those convs to the wrong write
            wq = [st.tile([ROWS, 4, XP], bf16, name=f"wq{i}") for i in range(6)]
            NVW = 5
            vq = [st.tile([ROWS, 4, XP], bf16, name=f"vq{i}") for i in range(NVW)]
            sq = [st.tile([ROWS, 4, XP], bf16, name=f"sq{i}") for i in range(2)]
            w0p = st.tile([ROWS, 2, XP], bf16)
            up = [st.tile([ROWS, 2, W], bf16, name=f"up{i}") for i in range(NPAIR)]
            stg = [st.tile([ROWS, W], f32, name=f"stg{i}") for i in range(3)]

            nc.vector.tensor_copy(ybf[:], y_sb[:])
            nc.vector.tensor_mul(out=m2_sb[:], in0=m_sb[:], in1=m_sb[:])
            nc.vector.tensor_scalar_mul(s_sb[:], y_sb[:], 2.0)
            nc.vector.memset(half[:], 0.5)
            nc.vector.memset(zr[:], 0.0)
            zp = st.tile([128, 2], bf16)
            nc.vector.memset(zp[:], 0.0)
            # zero the pad columns once; all later writes stay inside [2, 514)
            for t in (xs, *wq, *vq, *sq, w0p):
                nb = t.shape[1]
                nc.vector.tensor_copy(
                    t[:, :, 0:2], zp[:, None, :].to_broadcast((ROWS, nb, 2))
                )
                nc.vector.tensor_copy(
                    t[:, :, XP - 2 : XP], zp[:, None, :].to_broadcast((ROWS, nb, 2))
                )

            yb_tiles = {}

            def yb_tile(k):
                if k not in yb_tiles:
                    yb_tiles[k] = ybp.tile(
                        [ROWS, WM + 5], f32, tag="yb", name=f"yb{k}"
                    )
                return yb_tiles[k]

            def emit_zero_tail(k):
                nc.tensor.matmul(
                    yb_tile(k)[:, W : W + L], W_I, zr[:],
                    start=True, stop=False, skip_group_check=True,
                )

            def emit_A_band(k, l, u_ap):
                # matmul outs must not cross the PSUM bank boundary at col 512
                yb = yb_tile(k)
                if l == 0:
                    nc.tensor.matmul(
                        yb[:, 0:W], W_I, u_ap,
                        start=True, stop=False, skip_group_check=True,
                    )
                else:
                    nc.tensor.matmul(
                        yb[:, l:W], W_I, u_ap[:, 0 : W - l],
                        start=False, stop=False, skip_group_check=True,
                    )
                    nc.tensor.matmul(
                        yb[:, W : W + l], W_I, u_ap[:, W - l : W],
                        start=False, stop=(l == L - 1), skip_group_check=True,
                    )

            def u_engine(p):
                return nc.gpsimd if p in POOL_U_PAIRS else nc.vector

            # ---- preamble: u0 = (m*m)*y[shift] (one DVE/Pool op per pair),
            # x0 = m*y[shift] (one 28-band DVE op), then the A(0) block
            for p in range(NPAIR):
                eng = nc.gpsimd if p in (0, 2, 4, 6) else nc.vector
                eng.tensor_mul(
                    out=up[p][:],
                    in0=m2_sb[:, None, :].to_broadcast((ROWS, 2, W)),
                    in1=_win3(ybf, 2 * p, 2, W),
                )
            nc.vector.tensor_mul(
                out=xs[:, :, 2 : 2 + W],
                in0=m_sb[:, None, :].to_broadcast((ROWS, L, W)),
                in1=_win3(ybf, 0, L, W),
            )
            emit_zero_tail(0)
            for p in range(NPAIR):
                emit_A_band(0, 2 * p, up[p][:, 0, :])
                emit_A_band(0, 2 * p + 1, up[p][:, 1, :])

            # ---- iterations
            x2_tiles = {}

            def x2_tile(k, b):
                x2_tiles[(k, b)] = cp.tile([ROWS, W], f32, tag="x2", name=f"x2_{k}_{b}")
                return x2_tiles[(k, b)]

            def emit_conv_mms(x2, rhs_tile, idx, start, stop, sw_tile=None):
                # rhs_tile: [ROWS, nb, XP] holding the band at cols [2, 514)
                nc.tensor.matmul(
                    x2[:], W_C[1], rhs_tile[:, idx, 2 : 2 + W],
                    start=start, stop=False, skip_group_check=True,
                )
                if sw_tile is not None:
                    # wing taps fused: g3[0]*B @ (w<<1 + w>>1)
                    nc.tensor.matmul(
                        x2[:], W_C[0], sw_tile[:, idx, 2 : 2 + W],
                        start=False, stop=stop, skip_group_check=True,
                    )
                    return
                nc.tensor.matmul(
                    x2[:], W_C[0], rhs_tile[:, idx, 1 : 1 + W],
                    start=False, stop=False, skip_group_check=True,
                )
                nc.tensor.matmul(
                    x2[:], W_C[2], rhs_tile[:, idx, 3 : 3 + W],
                    start=False, stop=stop, skip_group_check=True,
                )

            def emit_copy_band(k, b, last):
                x2 = x2_tiles[(k, b)]
                if last:
                    sg = stg[b % 3]
                    nc.scalar.copy(sg[:], x2[:])
                    nc.sync.dma_start(out[b, :, :], sg[HALO : HALO + OUT_ROWS, :])
                else:
                    nc.scalar.copy(xs[:, b, 2 : 2 + W], x2[:])

            def emit_u_pair(k, p):
                # u_l = m * x_l (new xs) -> feeds the yb(k+1) A block
                u_engine(p).tensor_mul(
                    out=up[p][:],
                    in0=m_sb[:, None, :].to_broadcast((ROWS, 2, W)),
                    in1=xs[:, 2 * p : 2 * p + 2, 2 : 2 + W],
                )

            for k in range(n_iter):
                last = k == n_iter - 1
                yb = yb_tile(k)
                # boundary: conv(x) of split bands 0,1 needs no t0 -> PE
                # crosses the A->t0->v0 chain without idling
                for b in range(2):
                    emit_conv_mms(x2_tile(k, b), xs, b, start=True, stop=False)
                # B: t0 = s - 2*yb  (bf16 out)
                nc.vector.scalar_tensor_tensor(
                    out=t0_sb[:], in0=yb[:, 0:WM], scalar=-2.0,
                    in1=s_sb[:], op0=MUL, op1=ADD,
                )
                # quad 0: v in two halves; conv + copies interleaved
                nc.vector.tensor_mul(
                    out=vq[0][:, 0:2, 2 : 2 + W],
                    in0=mi[:, 0:2, :],
                    in1=_win3(t0_sb, 0, 2, W),
                )
                for b in range(2):
                    emit_conv_mms(x2_tiles[(k, b)], vq[0], b, start=False, stop=True)
                emit_copy_band(k, 0, last)
                emit_copy_band(k, 1, last)
                nc.vector.tensor_mul(
                    out=vq[0][:, 2:4, 2 : 2 + W],
                    in0=mi[:, 2:4, :],
                    in1=_win3(t0_sb, 2, 2, W),
                )
                nc.vector.tensor_add(
                    out=w0p[:, :, 2 : 2 + W],
                    in0=xs[:, 2:4, 2 : 2 + W],
                    in1=vq[0][:, 2:4, 2 : 2 + W],
                )
                for b in (2, 3):
                    emit_conv_mms(x2_tile(k, b), w0p, b - 2, start=True, stop=True)
                emit_copy_band(k, 2, last)
                emit_copy_band(k, 3, last)
                # all remaining v/w/sw upfront: the in-order DVE queue stays
                # two-plus quads ahead of the PE conv loop
                for q in range(1, 7):
                    nc.vector.tensor_mul(
                        out=vq[q % NVW][:, 0:4, 2 : 2 + W],
                        in0=mi[:, 4 * q : 4 * q + 4, :],
                        in1=_win3(t0_sb, 4 * q, 4, W),
                    )
                    nc.vector.tensor_add(
                        out=wq[q - 1][:, 0:4, 2 : 2 + W],
                        in0=xs[:, 4 * q : 4 * q + 4, 2 : 2 + W],
                        in1=vq[q % NVW][:, 0:4, 2 : 2 + W],
                    )
                    if q in WING_QUADS:
                        wb = wq[q - 1]
                        nc.vector.tensor_add(
                            out=sq[WING_QUADS.index(q)][:, 0:4, 2 : 2 + W],
                            in0=wb[:, 0:4, 1 : 1 + W],
                            in1=wb[:, 0:4, 3 : 3 + W],
                        )
                # conv loop (PE) + copies (ACT) + Pool u pairs
                for q in range(1, 7):
                    wb = wq[q - 1]
                    sb = sq[WING_QUADS.index(q)] if q in WING_QUADS else None
                    for b in range(4 * q, 4 * q + 4):
                        emit_conv_mms(
                            x2_tile(k, b), wb, b - 4 * q,
                            start=True, stop=True, sw_tile=sb,
                        )
                        emit_copy_band(k, b, last)
                    if not last and q <= 4:
                        # Pool u pairs, spread so GPSIMD starts early
                        for p in (2 * q - 2, 2 * q - 1):
                            if p in POOL_U_PAIRS:
                                emit_u_pair(k, p)
                if not last:
                    # remaining u pairs on DVE (after all v/w in queue order)
                    for p in range(NPAIR):
                        if p not in POOL_U_PAIRS:
                            emit_u_pair(k, p)
                    # the A(k+1) block: all identity matmuls at the very end
                    # of PE's program order; t0(k+1) follows the last one
                    emit_zero_tail(k + 1)
                    for p in range(NPAIR):
                        emit_A_band(k + 1, 2 * p, up[p][:, 0, :])
                        emit_A_band(k + 1, 2 * p + 1, up[p][:, 1, :])
                    # s' = 0.5*(s + t0) + y  == s + y - yb, on GPSIMD (reads
                    # only SBUF), needed only by t0(k+1)
                    nc.gpsimd.tensor_add(
                        out=stmp[:], in0=s_sb[:], in1=t0_sb[:]
                    )
                    nc.gpsimd.tensor_mul(
                        out=s_sb[:], in0=stmp[:],
                        in1=half[:, 0:1].to_broadcast((ROWS, WM)),
                    )
                    nc.gpsimd.tensor_add(
                        out=s_sb[:], in0=s_sb[:], in1=y_sb[:]
                    )

    _split_excess_waits(nc, max_w=1)
    return nc


def _host_inputs(y_1hw, mask2d):
    y2 = np.asarray(y_1hw, dtype=np.float32)[0]      # [512, 539]
    m2 = np.asarray(mask2d, dtype=np.float32)        # [512, 512]
    g3 = _gauss3(SIGMA)

    # Phi_sum / mi on the full grid (host precompute; Phi depends only on m)
    Phi = np.zeros((H, WM), dtype=np.float32)
    for l in range(L):
        Phi[:, l : l + W] += m2
    Phi = np.maximum(Phi, 1.0)
    invPhi = (1.0 / Phi).astype(np.float32)

    ident = np.eye(128, dtype=np.float32)

    in_maps = []
    for c in range(NCORES):
        rk = 64 * c - HALO
        y_slab = np.zeros((ROWS, WM), dtype=np.float32)
        m_slab = np.zeros((ROWS, W), dtype=np.float32)
        mi_slab = np.zeros((ROWS, L, W), dtype=np.float32)
        lo = max(0, -rk)              # first valid slab row
        hi = min(ROWS, H - rk)        # one past last valid slab row
        y_slab[lo:hi] = y2[rk + lo : rk + hi]
        m_slab[lo:hi] = m2[rk + lo : rk + hi]
        iv = invPhi[rk + lo : rk + hi]  # [vr, WM]
        for l in range(L):
            mi_slab[lo:hi, l, :] = m_slab[lo:hi] * iv[:, l : l + W]
        # banded 3-tap row-conv matrix, zeroed outside the valid row range
        B = np.zeros((128, 128), dtype=np.float32)
        for kk in range(-1, 2):
            for i in range(128):
                ip = i + kk
                if lo <= i < hi and lo <= ip < hi:
                    B[ip, i] = g3[kk + 1]
        wmats = np.zeros((128, 4, 128), dtype=np.float32)
        wmats[:, 0, :] = ident
        for t in range(3):
            wmats[:, 1 + t, :] = g3[t] * B
        in_maps.append(
            {
                "y_slab": y_slab,
                "m_slab": m_slab.astype(ml_dtypes.bfloat16),
                "mi_slab": mi_slab.astype(ml_dtypes.bfloat16),
                "wmats": wmats.astype(ml_dtypes.bfloat16),
            }
        )
    return in_maps


_NC_CACHE = {}


def _get_nc(dx, n_iter=N_ITER):
    key = (tuple(int(v) for v in dx), n_iter)
    if key not in _NC_CACHE:
        assert all(int(d) == i for i, d in enumerate(key[0])), (
            "kernel assumes dx[l] == l"
        )
        _NC_CACHE[key] = build_nc(n_iter)
    return _NC_CACHE[key]


def kernel(y_1hw, mask2d, phi_d_deg, s_nom, n_iter=N_ITER, trace=False):
    s = np.asarray(s_nom, dtype=np.float32)
    phi = float(np.asarray(phi_d_deg))
    dx, dy = _offsets(s, phi)
    assert (dy == 0).all(), "kernel assumes dy == 0 (row shifts unsupported)"
    nc = _get_nc(dx, n_iter)
    in_maps = _host_inputs(y_1hw, mask2d)
    res = run_bass_kernel_spmd(nc, in_maps, list(range(NCORES)), trace=trace)
    x_full = np.empty((1, L, H, W), dtype=np.float32)
    for c in range(NCORES):
        x_full[0, :, 64 * c : 64 * (c + 1), :] = res.results[c]["xout"]
    kernel.last_results = res
    return x_full
